# revision 34
# baseline (speedup 1.0000x reference)
"""Multi-head attention (b=2, l=2048, d_model=1024, h=16) on 8 trn2 NeuronCores.

Sharding: tensor-parallel over heads. Each core owns 2 heads: QKV projections
for its 128 channels (transposed layout), attention for its heads, and a
rank-128 partial of the output projection. The host sums the 8 partials and
adds b_o (the tensor-parallel all-reduce, done at gather time).

Design (all matmuls bf16, fp32 PSUM accumulate; ACT exp is the pacer):
  A: x arrives as batch-major contiguous bf16 DMAs under a PE warmup burst.
     V projected (transposed VT) for all tokens, PE-transposed into per-head
     natural-layout Vaug tiles with a ones column (softmax Z); then Q/K for
     batch 0. Biases are added by DVE at psum evacuation (free vs a copy).
     Projections alternate between the two psum rings for pipeline depth.
  B: flat stream of 128 (batch, 512-q-chunk, k-tile) periods. Per period:
     both heads' scoresT issued back to back on disjoint PE row groups
     (0-63/64-127 -> concurrent via tile_position), ONE [128,1024] exp over
     both heads (ACT ~1.1us paces the kernel), per-head PV accumulation
     [V_h|1].T @ exp into psum [65,512]. Scores psum double-buffered with S
     emitted two periods ahead so ACT never idles; PV at a block start is
     deferred past the next scores (its psum slot waits the previous block's
     evacuation). Q/K batch-1 projections and output-projection units stream
     into the PE/DVE shadow, kept away from block boundaries where DVE
     evacuates attn+Z (Z transposed via a tiny DRAM bounce -> 1/Z).
  C: out[tok,:] = sum_h (attnU_h.T @ Wo_h) * (1/Z_h): per-unit pair of
     64-contraction matmuls (row-group concurrent), per-partition 1/Z scales
     fused into the DVE evacuation (ACT helps in the tail, where units
     alternate between both psum rings for double pipeline depth).
     1/sqrt(dh) folded into Wq/bq on the host. Output partials written bf16
     oc-major (contiguous DMA rows); host sums in fp64.
"""
import sys
import types

import numpy as np

D_MODEL = 1024
H = 16
DH = 64
B = 2
L = 2048
BL = B * L            # 4096 tokens
NCORES = 8
NKT = D_MODEL // 128  # 8 feature tiles
TCH = 512             # phase-A token chunk
NCH = BL // TCH       # 8 chunks
QC = 512              # phase-B q chunk
NQC = L // QC         # 4 per batch
NKB = L // 128        # 16 k-tiles per batch
NG = BL // 128        # 32 global k-tile groups
VS = DH + 1           # per-k-tile Vaug cols: [V_h | 1]


def _register_ntff_hook():
    """Install the axon NTFF profiling hook module if the image lacks it."""
    if "antenv.axon_hooks" in sys.modules:
        return
    try:
        import antenv
        mod = types.ModuleType("antenv.axon_hooks")
        holder = {}
        mod.set_axon_ntff_profile_hook = lambda h: holder.__setitem__("h", h)
        mod.get_axon_ntff_profile_hook = lambda: holder.get("h")
        sys.modules["antenv.axon_hooks"] = mod
        antenv.axon_hooks = mod
        from trn_agent_boot.trn_boot import _ntff_profile_via_ctypes
        mod.set_axon_ntff_profile_hook(
            _ntff_profile_via_ctypes("/opt/axon/libaxon_pjrt.so")
        )
    except Exception:
        pass


_NC_CACHE = {}


def _build():
    if "nc" in _NC_CACHE:
        return _NC_CACHE["nc"]
    import concourse.bacc as bacc
    import concourse.tile as tile
    import concourse.mybir as mybir

    F32 = mybir.dt.float32
    BF16 = mybir.dt.bfloat16
    AF = mybir.ActivationFunctionType
    ALU = mybir.AluOpType

    nc = bacc.Bacc("TRN2", target_bir_lowering=False, debug=False)

    xT_d = nc.dram_tensor("xT", [B, D_MODEL, L], BF16, kind="ExternalInput").ap()
    wq_d = nc.dram_tensor("wq", [128, NKT * 128], BF16, kind="ExternalInput").ap()
    wk_d = nc.dram_tensor("wk", [128, NKT * 128], BF16, kind="ExternalInput").ap()
    wv_d = nc.dram_tensor("wv", [128, NKT * 128], BF16, kind="ExternalInput").ap()
    bq_d = nc.dram_tensor("bq", [128, 1], F32, kind="ExternalInput").ap()
    bk_d = nc.dram_tensor("bk", [128, 1], F32, kind="ExternalInput").ap()
    bv_d = nc.dram_tensor("bv", [128, 1], F32, kind="ExternalInput").ap()
    wo_d = nc.dram_tensor("wo", [128, D_MODEL], BF16, kind="ExternalInput").ap()
    id_d = nc.dram_tensor("ident", [128, 128], BF16, kind="ExternalInput").ap()
    out_d = nc.dram_tensor("out", [2, BL, 512], BF16, kind="ExternalOutput").ap()

    with tile.TileContext(nc) as tc:
        with (
            tc.tile_pool(name="weights", bufs=1) as wpool,
            tc.tile_pool(name="persist", bufs=1) as ppool,
            tc.tile_pool(name="expP", bufs=8) as epool,
            tc.tile_pool(name="oout", bufs=6) as opool,
            tc.tile_pool(name="zcb", bufs=2) as zpool,
            tc.tile_pool(name="dram", bufs=1, space="DRAM") as dpool,
            tc.tile_pool(name="ps", bufs=1, space="PSUM") as psp,
        ):
            id_t = wpool.tile([128, 128], BF16, tag="ident")
            nc.gpsimd.dma_start(id_t[:], id_d)
            wq_t = wpool.tile([128, NKT * 128], BF16, tag="wq")
            wk_t = wpool.tile([128, NKT * 128], BF16, tag="wk")
            wv_t = wpool.tile([128, NKT * 128], BF16, tag="wv")
            bq_t = wpool.tile([128, 1], F32, tag="bq")
            bk_t = wpool.tile([128, 1], F32, tag="bk")
            bv_t = wpool.tile([128, 1], F32, tag="bv")
            wo_t = wpool.tile([128, D_MODEL], BF16, tag="wo")
            for t, d in ((wq_t, wq_d), (wk_t, wk_d), (wv_t, wv_d),
                         (bq_t, bq_d), (bk_t, bk_d), (bv_t, bv_d),
                         (wo_t, wo_d)):
                nc.gpsimd.dma_start(t[:], d)

            xall = ppool.tile([128, NKT, BL], BF16, tag="xall")
            QT = ppool.tile([128, BL], BF16, tag="QT")
            KT = ppool.tile([128, BL], BF16, tag="KT")
            VT = ppool.tile([128, BL], BF16, tag="VT")
            Vaug = [ppool.tile([128, NG * VS], BF16, tag=f"vaug{h}",
                               name=f"vaug{h}") for h in range(2)]
            attnU = [ppool.tile([128, L], BF16, tag=f"attnU{b}",
                                name=f"attnU{b}") for b in range(B)]
            rz = [[ppool.tile([128, L // 128], F32, tag=f"rz{h}{b}",
                              name=f"rz{h}{b}") for b in range(B)]
                  for h in range(2)]
            scr = ppool.tile([1, 32], F32, tag="scr")
            zscr = dpool.tile([2, BL], F32, tag="zscr")

            for h in range(2):
                nc.vector.memset(Vaug[h][:], 1.0)

            # ---- warmup: lift HAM clock gate + preload exp table ----
            # warmup sized to cover the first x DMA wait (~8us cold)
            wu = psp.tile([128, 512], F32, tag="po", name="wu", bufs=2)
            for i in range(72):
                nc.tensor.matmul(wu[:, 0:128], id_t[:], id_t[:],
                                 start=(i == 0), stop=(i == 71))
            nc.scalar.activation(scr[:], wu[0:1, 0:32], AF.Exp)

            # ---- x DMAs: batch-major contiguous rows; batch 0 split in
            # halves so the first V chunk can start sooner ----
            for hf in range(2):
                for kt in range(NKT):
                    nc.sync.dma_start(
                        xall[:, kt, hf * 1024:(hf + 1) * 1024],
                        xT_d[0, kt * 128:(kt + 1) * 128,
                             hf * 1024:(hf + 1) * 1024],
                    )
            for kt in range(NKT):
                nc.sync.dma_start(
                    xall[:, kt, L:2 * L], xT_d[1, kt * 128:(kt + 1) * 128, :]
                )

            # ---- phase A0: V for all chunks + transposes; Q/K for batch 0 ----
            for c in range(NCH):
                csl = slice(c * TCH, (c + 1) * TCH)
                psv = psp.tile([128, TCH], F32, tag=("po" if c % 2 else "scb"),
                               name="psv", bufs=2)
                for kt in range(NKT):
                    nc.tensor.matmul(
                        psv[:], wv_t[:, kt * 128:(kt + 1) * 128],
                        xall[:, kt, csl], start=(kt == 0),
                        stop=(kt == NKT - 1),
                    )
                nc.vector.tensor_scalar_add(VT[:, csl], psv[:], bv_t[:, 0:1])
                for g in range(c * (TCH // 128), (c + 1) * (TCH // 128)):
                    tp = psp.tile([128, 128], BF16, tag="scb", name="tp", bufs=2)
                    nc.tensor.transpose(
                        tp[:], VT[:, g * 128:(g + 1) * 128], id_t[:]
                    )
                    for h in range(2):
                        nc.vector.tensor_copy(
                            Vaug[h][:, g * VS:g * VS + DH],
                            tp[:, h * DH:(h + 1) * DH],
                        )
            for w_t, b_t, dst, ptag in ((wq_t, bq_t, QT, "scb"),
                                        (wk_t, bk_t, KT, "po")):
                for c in range(NCH // 2):
                    csl = slice(c * TCH, (c + 1) * TCH)
                    psq = psp.tile([128, TCH], F32, tag=ptag, name="psq", bufs=2)
                    for kt in range(NKT):
                        nc.tensor.matmul(
                            psq[:], w_t[:, kt * 128:(kt + 1) * 128],
                            xall[:, kt, csl], start=(kt == 0),
                            stop=(kt == NKT - 1),
                        )
                    nc.vector.tensor_scalar_add(dst[:, csl], psq[:],
                                                b_t[:, 0:1])

            # ---- shadow work: Q/K batch-1 projections, then O-proj units ----
            qk_items = []
            for w_t, b_t, dst in ((wq_t, bq_t, QT), (wk_t, bk_t, KT)):
                for c in range(NCH // 2, NCH):
                    csl = slice(c * TCH, (c + 1) * TCH)
                    st = {}

                    def mk_mm(kt, st=st, w_t=w_t, csl=csl):
                        def f():
                            if kt == 0:
                                st["ps"] = psp.tile([128, TCH], F32,
                                                    tag="po", name="sps", bufs=2)
                            nc.tensor.matmul(
                                st["ps"][:],
                                w_t[:, kt * 128:(kt + 1) * 128],
                                xall[:, kt, csl], start=(kt == 0),
                                stop=(kt == NKT - 1),
                            )
                        return f

                    def mk_ev(st=st, dst=dst, b_t=b_t, csl=csl):
                        def f():
                            nc.vector.tensor_scalar_add(dst[:, csl],
                                                        st["ps"][:],
                                                        b_t[:, 0:1])
                        return f

                    for kt in range(NKT):
                        qk_items.append(mk_mm(kt))
                    qk_items.append(mk_ev())

            def emit_unit(b, t, oc, tail=False, ptag="po"):
                """One output-projection unit: 128 tokens x 512 out-cols,
                both heads on disjoint PE row groups, deferred 1/Z scales."""
                lrsl = slice(t * 128, (t + 1) * 128)
                osl = slice(oc * 512, (oc + 1) * 512)
                ps0 = psp.tile([128, 512], F32, tag=ptag, name="ps0", bufs=2)
                ps1 = psp.tile([128, 512], F32, tag=ptag, name="ps1", bufs=2)
                nc.tensor.matmul(ps0[:], attnU[b][0:64, lrsl],
                                 wo_t[0:64, osl], start=True, stop=True)
                nc.tensor.matmul(ps1[:], attnU[b][64:128, lrsl],
                                 wo_t[64:128, osl], start=True, stop=True)
                tmp = opool.tile([128, 512], F32, tag="tmp", name="tmp")
                if tail:
                    nc.scalar.activation(tmp[:], ps0[:], AF.Copy,
                                         scale=rz[0][b][:, t:t + 1])
                else:
                    nc.vector.tensor_scalar_mul(tmp[:], ps0[:],
                                                rz[0][b][:, t:t + 1])
                ot = opool.tile([128, 512], BF16, tag="ot", name="ot")
                nc.vector.scalar_tensor_tensor(
                    ot[:], ps1[:], rz[1][b][:, t:t + 1], tmp[:],
                    op0=ALU.mult, op1=ALU.add,
                )
                nc.sync.dma_start(
                    out_d[oc, b * L + t * 128:b * L + (t + 1) * 128, :], ot[:])

            # ---- phase B: flat k-tile stream over all (b, qc) blocks ----
            blocks = [(b, qc) for b in range(B) for qc in range(NQC)]
            allS = [(b, qc, kt) for (b, qc) in blocks for kt in range(NKB)]
            sc_of = {}

            def emit_S(i):
                b, qc, kt = allS[i]
                sc = psp.tile([128, 1024], F32, tag="scb", name="sc", bufs=2)
                q0 = b * L + qc * QC
                ksl = slice(b * L + kt * 128, b * L + (kt + 1) * 128)
                for h in range(2):
                    hs = slice(h * 64, (h + 1) * 64)
                    nc.tensor.matmul(sc[:, h * 512:(h + 1) * 512],
                                     KT[hs, ksl], QT[hs, q0:q0 + QC],
                                     start=True, stop=True)
                sc_of[i] = sc

            emit_S(0)
            emit_S(1)
            unit_q = []
            pv_pending = []
            z_pending = []
            pv_started = set()
            pv = None
            for i, (b, qc, kt) in enumerate(allS):
                if kt == 0:
                    pv = psp.tile([128, 1024], F32, tag="pv", name="pv", bufs=1)
                sc = sc_of.pop(i)
                ex = epool.tile([128, 1024], BF16, tag="ex", name="ex")
                nc.scalar.activation(ex[:], sc[:], AF.Exp)
                if i + 2 < len(allS):
                    emit_S(i + 2)
                g = b * NKB + kt

                def emit_PV(pv=pv, g=g, ex=ex, kt=kt, bqc=(b, qc)):
                    first = bqc not in pv_started
                    pv_started.add(bqc)
                    for h in range(2):
                        nc.tensor.matmul(
                            pv[0:VS, h * 512:(h + 1) * 512],
                            Vaug[h][:, g * VS:(g + 1) * VS],
                            ex[:, h * 512:(h + 1) * 512],
                            start=first, stop=(kt == NKB - 1),
                        )

                # At a block start PV(0)/PV(1) wait on the previous block's
                # psum evacuation (DVE); defer them past the next scores so
                # the exp chain never queues behind them on the in-order PE.
                if kt >= 2:
                    while pv_pending:
                        pv_pending.pop(0)()
                    emit_PV()
                # shadow: drain b1 Q/K projections first, then O-proj units.
                # Units carry ~1.4us of DVE evacuation, which also spikes at
                # block boundaries (attn/Z evac + reciprocal) — keep units
                # away from the boundary so the in-order PE queue never
                # blocks on a psum slot behind a DVE backlog.
                if qk_items:
                    qk_items.pop(0)()
                    if qk_items:
                        qk_items.pop(0)()
                elif unit_q and 1 < kt < NKB - 1:
                    emit_unit(*unit_q.pop(0))
                if kt < 2:
                    pv_pending.append(emit_PV)
                if kt == NKB - 1:
                    # Z row -> 1/Z per 128-token tile (via tiny DRAM bounce).
                    # The psum->SBUF copy rides ACT's idle boundary gap; for
                    # every block but the last it's deferred into the next
                    # block so it never delays the exp chain.
                    def z_chain(pv=pv, b=b, qc=qc):
                        zsl = slice(b * L + qc * QC, b * L + (qc + 1) * QC)
                        zsb = zpool.tile([1, 1024], F32, tag="zsb", name="zsb")
                        nc.vector.tensor_copy(zsb[:], pv[DH:DH + 1, 0:1024])
                        nc.sync.dma_start(zscr[0:2, zsl], zsb[:])
                        for h in range(2):
                            zc = zpool.tile([128, QC // 128], F32, tag="zc",
                                            name="zc")
                            nc.sync.dma_start(
                                zc[:],
                                zscr[h, zsl.start:zsl.stop]
                                .rearrange("(c p) -> p c", p=128),
                            )
                            nc.vector.reciprocal(
                                rz[h][b][:, qc * (QC // 128):
                                         (qc + 1) * (QC // 128)], zc[:])

                    def z_fast(pv=pv, b=b, qc=qc):
                        # latency-critical final block: flip the Z row into
                        # psum columns through the PE instead of the DRAM
                        # bounce; ACT (idle in the tail) reads the psum row.
                        zsb = zpool.tile([1, 1024], BF16, tag="zsb2",
                                         name="zsb2")
                        nc.scalar.copy(zsb[:], pv[DH:DH + 1, 0:1024])
                        zps = psp.tile([128, 8, 2], BF16, tag="pv",
                                       name="zps", bufs=1)
                        for j in range(8):
                            nc.tensor.transpose(
                                zps[:, j, 0:1],
                                zsb[0:1, j * 128:(j + 1) * 128],
                                id_t[0:1, 0:1],
                            )
                        for h in range(2):
                            nc.vector.reciprocal(
                                rz[h][b][:, qc * (QC // 128):
                                         (qc + 1) * (QC // 128)],
                                zps[:, h * 4:(h + 1) * 4, 0])
                    if i == len(allS) - 1:
                        z_fast()
                    else:
                        z_chain()
                    qsl = slice(qc * QC, (qc + 1) * QC)
                    for h in range(2):
                        nc.vector.tensor_copy(
                            attnU[b][h * 64:(h + 1) * 64, qsl],
                            pv[0:DH, h * 512:(h + 1) * 512],
                        )
                    unit_q += [(b, qc * (QC // 128) + t, oc)
                               for t in range(QC // 128) for oc in range(2)]

            # ---- tail: leftover output-projection units. Scores psum is
            # free now, so alternate units between the po and scb rings for
            # twice the pipeline depth.
            for i, u in enumerate(unit_q):
                emit_unit(*u, tail=True, ptag=("po" if i % 2 == 0 else "scb"))

    nc.compile()
    _NC_CACHE["nc"] = nc
    return nc


def _shard_inputs(x, W_qkv, b_qkv, W_o):
    import ml_dtypes
    bf16 = ml_dtypes.bfloat16
    xT = np.ascontiguousarray(
        x.reshape(BL, D_MODEL).T.astype(np.float32)).astype(bf16)
    ident = np.eye(128, dtype=np.float32).astype(bf16)

    xT = np.ascontiguousarray(
        xT.reshape(D_MODEL, B, L).transpose(1, 0, 2))  # [B, D_MODEL, L]

    def lhsT_layout(w):
        # [D_MODEL, 128] -> [128, NKT*128] with [p, kt*128+ch] = w[kt*128+p, ch]
        return np.ascontiguousarray(
            w.reshape(NKT, 128, 128).transpose(1, 0, 2).reshape(128, NKT * 128)
            .astype(np.float32)).astype(bf16)

    in_maps = []
    for c in range(NCORES):
        cs = slice(c * 128, (c + 1) * 128)
        wq = W_qkv[:, cs] * 0.125
        wk = W_qkv[:, D_MODEL:][:, cs]
        wv = W_qkv[:, 2 * D_MODEL:][:, cs]
        in_maps.append({
            "xT": xT,
            "wq": lhsT_layout(wq), "wk": lhsT_layout(wk), "wv": lhsT_layout(wv),
            "bq": np.ascontiguousarray(
                (b_qkv[cs] * 0.125).astype(np.float32)).reshape(128, 1),
            "bk": np.ascontiguousarray(
                b_qkv[D_MODEL:][cs].astype(np.float32)).reshape(128, 1),
            "bv": np.ascontiguousarray(
                b_qkv[2 * D_MODEL:][cs].astype(np.float32)).reshape(128, 1),
            "wo": np.ascontiguousarray(
                W_o[cs, :].astype(np.float32)).astype(bf16),
            "ident": ident,
        })
    return in_maps


def _run(inputs, trace=False, tmpdir=None):
    from concourse.bass_utils import run_bass_kernel_spmd

    _register_ntff_hook()
    nc = _build()
    in_maps = _shard_inputs(
        np.asarray(inputs["x"], dtype=np.float32),
        np.asarray(inputs["W_qkv"], dtype=np.float32),
        np.asarray(inputs["b_qkv"], dtype=np.float32),
        np.asarray(inputs["W_o"], dtype=np.float32),
    )
    res = run_bass_kernel_spmd(nc, in_maps, core_ids=list(range(NCORES)),
                               trace=trace, tmpdir=tmpdir)
    partial = np.zeros((BL, D_MODEL), dtype=np.float64)
    for c in range(NCORES):
        o = np.asarray(res.results[c]["out"]).astype(np.float64)
        partial += o.transpose(1, 0, 2).reshape(BL, D_MODEL)
    out = (partial + np.asarray(inputs["b_o"], dtype=np.float64)).astype(np.float32)
    return out.reshape(B, L, D_MODEL), res


def kernel(**inputs) -> np.ndarray:
    out, _ = _run(inputs, trace=False)
    return out
